# revision 17
# baseline (speedup 1.0000x reference)
"""Trainium2 Bass kernel for nn_MultiHeadAttention_38611755991513.

Reference computation (B=2, D=1024, L=2048, H=16, DK=64):
    q/k/v = conv1d(kernel=1) projections of query [B, D, L]
    att   = softmax(mask(q^T k / sqrt(DK)))   with key-only mask [B, 1, L]
    out   = Wo @ (att @ v heads recombined) + bo

Sharding: 32 (batch, head) pairs -> 4 heads (one batch) per core.
Each core computes its 4 heads' attention plus the partial O-projection
(Wo columns for its heads); the host sums the 4 partials per batch.

Key optimization: the mask is key-only, so masked keys are compacted away
on the host (the kernel only ever sees valid keys, zero-padded to a common
length L_c across batches; padded keys get zeroed V rows and a zeroed
ones-column so they contribute nothing to either the attention numerator
or the softmax denominator).

Layout: scores are computed transposed (S^T[k, q]) so that exp(S^T) is
directly the moving operand of the att@v matmul; the softmax denominator
comes for free as a 65th "ones" column of the V operand.
"""

import sys

sys.path.insert(0, "/opt/trn_rl_repo")

import numpy as np
import ml_dtypes

import concourse.bass as bass
import concourse.tile as tile
from concourse import bacc, mybir
from concourse.bass_utils import run_bass_kernel_spmd

B, D, L, H = 2, 1024, 2048, 16
DK = 64
NCORES = 8
HPC = 4              # heads per core
DH = HPC * DK        # 256 head-dims per core
KT = D // 128        # 8 contraction tiles for the projections
BF16 = mybir.dt.bfloat16
F32 = mybir.dt.float32
NPBF16 = ml_dtypes.bfloat16

TRACE = False            # set True (e.g. from test.py) to capture a HW profile
LAST_EXEC_NS = None
LAST_RESULTS = None


def _chunks(total, size):
    out = []
    s = 0
    while s < total:
        w = min(size, total - s)
        out.append((s, w))
        s += w
    return out


def _build(L_c):
    """Build + compile the per-core Bass program for compacted key length L_c."""
    nc = bacc.Bacc("TRN2", debug=False, num_devices=NCORES)
    mts = _chunks(L_c, 128)
    MT = len(mts)
    EXP = mybir.ActivationFunctionType.Exp

    xb_d = nc.declare_dram_parameter("xb", [D, L], BF16, isOutput=False)
    xk_d = nc.declare_dram_parameter("xk", [D, L_c], BF16, isOutput=False)
    vo_d = nc.declare_dram_parameter("vones", [L_c, HPC, 1], F32, isOutput=False)
    wq_d = nc.declare_dram_parameter("wq", [D, DH], BF16, isOutput=False)
    wk_d = nc.declare_dram_parameter("wk", [D, DH], BF16, isOutput=False)
    wv_d = nc.declare_dram_parameter("wv", [D, DH], BF16, isOutput=False)
    wo_d = nc.declare_dram_parameter("wo", [DH, D], BF16, isOutput=False)
    bq_d = nc.declare_dram_parameter("bq", [2, 128, 1], F32, isOutput=False)
    bk_d = nc.declare_dram_parameter("bk", [2, 128, 1], F32, isOutput=False)
    bv_d = nc.declare_dram_parameter("bv", [2, 128, 1], F32, isOutput=False)
    out_d = nc.declare_dram_parameter("out", [D, L], BF16, isOutput=True)

    from contextlib import ExitStack
    with tile.TileContext(nc) as tc, ExitStack() as ctx:
        pers = ctx.enter_context(tc.tile_pool(name="pers", bufs=1))

        def ptile(shape, dtype, name):
            return pers.tile(shape, dtype, tag=name, name=name)

        # persistent SBUF tensors
        xk_t = [ptile([128, L_c], BF16, f"xk{i}") for i in range(KT)]
        xb_t = [ptile([128, L], BF16, f"xb{i}") for i in range(KT)]
        wq_t = [ptile([128, DH], BF16, f"wq{i}") for i in range(KT)]
        wk_t = [ptile([128, DH], BF16, f"wk{i}") for i in range(KT)]
        wv_t = [ptile([128, DH], BF16, f"wv{i}") for i in range(KT)]
        wo_t = [ptile([128, D], BF16, f"wo{i}") for i in range(2)]
        bq_t = [ptile([128, 1], F32, f"bq{i}") for i in range(2)]
        bk_t = [ptile([128, 1], F32, f"bk{i}") for i in range(2)]
        bv_t = [ptile([128, 1], F32, f"bv{i}") for i in range(2)]
        vo_t = [ptile([mw, HPC, 1], F32, f"vo{mt}") for mt, (ms, mw) in enumerate(mts)]
        q_t = [ptile([128, L], BF16, f"q{i}") for i in range(2)]
        k_t = [ptile([128, L_c], BF16, f"k{i}") for i in range(2)]
        z_t = [ptile([128, L], BF16, f"z{i}") for i in range(2)]
        va_t = [ptile([mw, HPC, 65], BF16, f"va{mt}") for mt, (ms, mw) in enumerate(mts)]

        # input DMAs, critical-path order: the K projection chain consumes
        # (xk_i, wk_i) pairs in i order, so interleave those; then wq and the
        # first 512-column slice of xb (all the first Q chain needs), then the
        # rest of xb, then the late-use tensors (wv, vones, wo).
        for i in range(KT):
            nc.sync.dma_start(xk_t[i][:], xk_d[i * 128:(i + 1) * 128, :])
            nc.sync.dma_start(wk_t[i][:], wk_d[i * 128:(i + 1) * 128, :])
        for i in range(2):
            nc.sync.dma_start(bk_t[i][:], bk_d[i])
            nc.sync.dma_start(bq_t[i][:], bq_d[i])
            nc.sync.dma_start(bv_t[i][:], bv_d[i])
        for i in range(KT):
            nc.sync.dma_start(wq_t[i][:], wq_d[i * 128:(i + 1) * 128, :])
            nc.sync.dma_start(xb_t[i][:, 0:512], xb_d[i * 128:(i + 1) * 128, 0:512])
        for js in (512, 1024, 1536):
            for i in range(KT):
                nc.sync.dma_start(
                    xb_t[i][:, js:js + 512], xb_d[i * 128:(i + 1) * 128, js:js + 512]
                )
        for i in range(KT):
            nc.sync.dma_start(wv_t[i][:], wv_d[i * 128:(i + 1) * 128, :])
        for mt, (ms, mw) in enumerate(mts):
            nc.sync.dma_start(vo_t[mt][:], vo_d[ms:ms + mw])
        for i in range(2):
            nc.sync.dma_start(wo_t[i][:], wo_d[i * 128:(i + 1) * 128, :])

        with (
            tc.tile_pool(name="psA", bufs=2, space="PSUM") as pa,
            tc.tile_pool(name="psY", bufs=2, space="PSUM") as pb,
            tc.tile_pool(name="psO", bufs=2, space="PSUM") as pox,
            tc.tile_pool(name="pexp", bufs=3 * MT) as pp,
            tc.tile_pool(name="osb", bufs=3) as po,
            tc.tile_pool(name="small", bufs=3) as psm,
        ):
            qblocks = _chunks(L, 1024)

            def k_chain(kt, ns, nw):
                kp = pox.tile([128, nw], F32, tag="po", name=f"kp{kt}_{ns}")
                for kk in range(KT):
                    nc.tensor.matmul(
                        kp[:],
                        wk_t[kk][:, kt * 128:(kt + 1) * 128],
                        xk_t[kk][:, ns:ns + nw],
                        start=(kk == 0), stop=(kk == KT - 1),
                    )
                nc.vector.tensor_scalar_add(k_t[kt][:, ns:ns + nw], kp[:], bk_t[kt][:])

            def q_chain(qs, kt, js, jw):
                qp = pox.tile([128, jw], F32, tag="po", name=f"qp{kt}_{qs + js}")
                for kk in range(KT):
                    nc.tensor.matmul(
                        qp[:],
                        wq_t[kk][:, kt * 128:(kt + 1) * 128],
                        xb_t[kk][:, qs + js:qs + js + jw],
                        start=(kk == 0), stop=(kk == KT - 1),
                    )
                nc.vector.tensor_scalar_add(q_t[kt][:, qs + js:qs + js + jw], qp[:], bq_t[kt][:])

            def v_chain(mt):
                ms, mw = mts[mt]
                vp = pox.tile([mw, DH], F32, tag="po", name=f"vp{mt}")
                for kk in range(KT):
                    nc.tensor.matmul(
                        vp[:],
                        xk_t[kk][:, ms:ms + mw],
                        wv_t[kk][:],
                        start=(kk == 0), stop=(kk == KT - 1),
                    )
                for h in range(HPC):
                    nc.vector.tensor_copy(va_t[mt][:, h, 0:64], vp[:, h * 64:(h + 1) * 64])
                nc.vector.tensor_copy(va_t[mt][:, :, 64:65], vo_t[mt][:])

            def o_chunk(qs, m8, js, jw):
                op = pox.tile([128, jw], F32, tag="po", name=f"o{qs}_{m8}_{js}")
                for kt in range(2):
                    nc.tensor.matmul(
                        op[:],
                        wo_t[kt][:, m8 * 128:(m8 + 1) * 128],
                        z_t[kt][:, qs + js:qs + js + jw],
                        start=(kt == 0), stop=(kt == 1),
                    )
                ob = po.tile([128, jw], BF16, tag="ob", name=f"ob{qs}_{m8}_{js}")
                nc.vector.tensor_copy(ob[:], op[:])
                nc.sync.dma_start(out_d[m8 * 128:(m8 + 1) * 128, qs + js:qs + js + jw], ob[:])

            # minimal prologue: just enough K/Q for head 0's first scores
            kchunks = _chunks(L_c, 512)
            k_chain(0, *kchunks[0])
            q_chain(0, 0, 0, 512)
            q_chain(0, 0, 512, 512)

            # ---- software-pipelined attention, head-PAIR phases ----
            # The two heads of a ptile row-pack the PE array: their K=64 score
            # matmuls are emitted strictly alternating (rows 0-63 vs 64-127)
            # so the array streams both concurrently (~2x). The previous
            # pair's att@v chains + projections/O-chunks fill remaining PE
            # slots while ACT streams the exps.
            def y_head(h, qs, qw, p_tiles, yq):
                # enqueue one head's y chains (js-interleaved, 2 PSUM banks)
                yps = {}

                def y_mt(mt):
                    if mt == 0:
                        for js, jw in _chunks(qw, 512):
                            yps[js] = pb.tile([65, jw], F32, tag="y", name=f"y{qs}_{h}_{js}")
                    for js, jw in _chunks(qw, 512):
                        nc.tensor.matmul(
                            yps[js],
                            va_t[mt][:, h, :],
                            p_tiles[mt][:, js:js + jw],
                            start=(mt == 0), stop=(mt == MT - 1),
                        )

                def finish():
                    pt, off = h // 2, (h % 2) * 64
                    for js, jw in _chunks(qw, 512):
                        yp = yps[js]
                        rt = psm.tile([1, jw], F32, tag="rrow", name=f"rt{qs}_{h}_{js}")
                        nc.vector.tensor_copy(rt[:], yp[64:65, :])
                        rc = psm.tile([1, jw], F32, tag="recip", name=f"rc{qs}_{h}_{js}")
                        nc.vector.reciprocal_approx_fast(rc[:], rt[:])
                        rb = psm.tile([64, jw], F32, tag="rb", name=f"rb{qs}_{h}_{js}")
                        nc.gpsimd.partition_broadcast(rb[:], rc[:])
                        zsl = z_t[pt][off:off + 64, qs + js:qs + js + jw]
                        nc.vector.tensor_mul(zsl, yp[0:64, :], rb[:])
                        nc.vector.tensor_scalar_add(zsl, zsl, bv_t[pt][off:off + 64, :])

                for mt in range(MT):
                    yq.append(lambda mt=mt: y_mt(mt))
                yq.append(finish)

            fillers = []   # (cost, emit) pairs
            fi = 0

            def pop_fillers(budget):
                nonlocal fi
                while budget > 0 and fi < len(fillers):
                    cost, emit = fillers[fi]
                    emit()
                    fi += 1
                    budget -= cost
                return budget

            for ns, nw in kchunks[1:]:
                fillers.append((8, lambda ns=ns, nw=nw: k_chain(0, ns, nw)))
            for ns, nw in kchunks:
                fillers.append((8, lambda ns=ns, nw=nw: k_chain(1, ns, nw)))
            for js, jw in _chunks(1024, 512):
                fillers.append((8, lambda js=js, jw=jw: q_chain(0, 1, js, jw)))
            for kt in range(2):
                for js, jw in _chunks(1024, 512):
                    fillers.append((8, lambda kt=kt, js=js, jw=jw: q_chain(1024, kt, js, jw)))

            qblocks = _chunks(L, 1024)
            yq = []       # pending y work units of the previous pair

            for qi, (qs, qw) in enumerate(qblocks):
                for hp in range(2):
                    hA, hB = 2 * hp, 2 * hp + 1
                    off_pairs = ((0, hA), (64, hB))
                    first_phase = (qi, hp) == (0, 0)
                    if (qi, hp) == (1, 1):
                        # z of block 0 completed during the previous phase:
                        # its O-projection chunks become filler work now
                        pqs, pqw = qblocks[0]
                        for m8 in range(8):
                            for js, jw in _chunks(pqw, 512):
                                fillers.append((2, lambda pqs=pqs, m8=m8, js=js, jw=jw: o_chunk(pqs, m8, js, jw)))
                    pA, pB = [], []
                    for mt, (ms, mw) in enumerate(mts):
                        sps = [
                            pa.tile([mw, qw], F32, tag="wide", name=f"s{qs}_{h}_{mt}")
                            for off, h in off_pairs
                        ]
                        for js, jw in _chunks(qw, 512):
                            for (off, h), sp in zip(off_pairs, sps):
                                nc.tensor.matmul(
                                    sp[:, js:js + jw],
                                    k_t[hp][off:off + 64, ms:ms + mw],
                                    q_t[hp][off:off + 64, qs + js:qs + js + jw],
                                    start=True, stop=True,
                                )
                        for (off, h), sp in zip(off_pairs, sps):
                            px = pp.tile([mw, qw], BF16, tag="p", name=f"p{qs}_{h}_{mt}")
                            nc.scalar.activation(px[:], sp[:], EXP)
                            (pA if h == hA else pB).append(px)
                        # drain previous pair's y work + filler budget
                        for _ in range(3):
                            if yq:
                                yq.pop(0)()
                        if first_phase:
                            pop_fillers(8)
                            v_chain(mt)
                        else:
                            pop_fillers(4)
                    while yq:
                        yq.pop(0)()
                    yq = []
                    y_head(hA, qs, qw, pA, yq)
                    y_head(hB, qs, qw, pB, yq)

            # drain the final pair's y work, remaining fillers, final O block
            while yq:
                yq.pop(0)()
            while fi < len(fillers):
                pop_fillers(1000)
            qs, qw = qblocks[-1]
            for m8 in range(8):
                for js, jw in _chunks(qw, 512):
                    o_chunk(qs, m8, js, jw)

    nc.compile()
    return nc


_NC_CACHE = {}


def _get_nc(L_c):
    if L_c not in _NC_CACHE:
        _NC_CACHE[L_c] = _build(L_c)
    return _NC_CACHE[L_c]


def _install_ntff_hook():
    """Synthesize antenv.axon_hooks (missing in this image) so trace=True works."""
    import types

    if "antenv.axon_hooks" in sys.modules:
        return
    try:
        if "/root/.axon_site" not in sys.path:
            sys.path.insert(0, "/root/.axon_site")
        from trn_agent_boot.trn_boot import _ntff_profile_via_ctypes

        hook = _ntff_profile_via_ctypes("/opt/axon/libaxon_pjrt.so")
        mod = types.ModuleType("antenv.axon_hooks")
        mod.get_axon_ntff_profile_hook = lambda: hook
        import antenv  # noqa: F401

        sys.modules["antenv.axon_hooks"] = mod
    except Exception:
        pass


def kernel(query, att_mask, Wq, bq, Wk, bk, Wv, bv, Wo, bo):
    global LAST_EXEC_NS, LAST_RESULTS
    query = np.asarray(query, dtype=np.float32)
    mask = np.asarray(att_mask).astype(bool).reshape(B, L)
    Wq, bq = np.asarray(Wq, np.float32), np.asarray(bq, np.float32)
    Wk, bk = np.asarray(Wk, np.float32), np.asarray(bk, np.float32)
    Wv, bv = np.asarray(Wv, np.float32), np.asarray(bv, np.float32)
    Wo, bo = np.asarray(Wo, np.float32), np.asarray(bo, np.float32)

    valid = [np.nonzero(mask[b])[0] for b in range(B)]
    L_c = max(len(v) for v in valid)
    out = np.empty((B, D, L), np.float32)
    if L_c == 0:
        out[:] = bo[None, :, None]
        return out

    scale = np.float32(1.0 / np.sqrt(DK))
    # per-batch compacted keys + validity column
    xk_b, vones_b, xb_b = [], [], []
    for b in range(B):
        idx = valid[b]
        xk = np.zeros((D, L_c), np.float32)
        xk[:, :len(idx)] = query[b][:, idx]
        xk_b.append(xk.astype(NPBF16))
        vo = np.zeros((L_c, HPC, 1), np.float32)
        vo[:len(idx)] = 1.0
        vones_b.append(vo)
        xb_b.append(query[b].astype(NPBF16))

    in_maps = []
    for c in range(NCORES):
        b, g = divmod(c, NCORES // B)
        sl = slice(g * DH, (g + 1) * DH)
        in_maps.append({
            "xb": xb_b[b],
            "xk": xk_b[b],
            "vones": vones_b[b],
            "wq": np.ascontiguousarray((Wq[sl, :] * scale).T).astype(NPBF16),
            "wk": np.ascontiguousarray(Wk[sl, :].T).astype(NPBF16),
            "wv": np.ascontiguousarray(Wv[sl, :].T).astype(NPBF16),
            "wo": np.ascontiguousarray(Wo[:, sl].T).astype(NPBF16),
            "bq": (bq[sl] * scale).reshape(2, 128, 1).astype(np.float32),
            "bk": bk[sl].reshape(2, 128, 1).astype(np.float32),
            "bv": bv[sl].reshape(2, 128, 1).astype(np.float32),
        })

    nc = _get_nc(L_c)
    if TRACE:
        _install_ntff_hook()
    res = run_bass_kernel_spmd(nc, in_maps, core_ids=list(range(NCORES)), trace=TRACE)
    LAST_EXEC_NS = res.exec_time_ns
    LAST_RESULTS = res

    parts = [res.results[c]["out"] for c in range(NCORES)]
    for b in range(B):
        if len(valid[b]) == 0:
            out[b] = bo[:, None]
        else:
            acc = parts[4 * b].astype(np.float32)
            for g in range(1, 4):
                acc = acc + parts[4 * b + g]
            out[b] = acc + bo[:, None]
    return out


# revision 20
# speedup vs baseline: 1.0289x; 1.0289x over previous
"""Trainium2 Bass kernel for nn_MultiHeadAttention_38611755991513.

Reference computation (B=2, D=1024, L=2048, H=16, DK=64):
    q/k/v = conv1d(kernel=1) projections of query [B, D, L]
    att   = softmax(mask(q^T k / sqrt(DK)))   with key-only mask [B, 1, L]
    out   = Wo @ (att @ v heads recombined) + bo

Sharding: 32 (batch, head) pairs -> 4 heads (one batch) per core.
Each core computes its 4 heads' attention plus the partial O-projection
(Wo columns for its heads); the host sums the 4 partials per batch.

Key optimization: the mask is key-only, so masked keys are compacted away
on the host (the kernel only ever sees valid keys, zero-padded to a common
length L_c across batches; padded keys get zeroed V rows and a zeroed
ones-column so they contribute nothing to either the attention numerator
or the softmax denominator).

Layout: scores are computed transposed (S^T[k, q]) so that exp(S^T) is
directly the moving operand of the att@v matmul; the softmax denominator
comes for free as a 65th "ones" column of the V operand.
"""

import sys

sys.path.insert(0, "/opt/trn_rl_repo")

import numpy as np
import ml_dtypes

import concourse.bass as bass
import concourse.tile as tile
from concourse import bacc, mybir
from concourse.bass_utils import run_bass_kernel_spmd

B, D, L, H = 2, 1024, 2048, 16
DK = 64
NCORES = 8
HPC = 4              # heads per core
DH = HPC * DK        # 256 head-dims per core
KT = D // 128        # 8 contraction tiles for the projections
BF16 = mybir.dt.bfloat16
F32 = mybir.dt.float32
NPBF16 = ml_dtypes.bfloat16

TRACE = False            # set True (e.g. from test.py) to capture a HW profile
LAST_EXEC_NS = None
LAST_RESULTS = None


def _chunks(total, size):
    out = []
    s = 0
    while s < total:
        w = min(size, total - s)
        out.append((s, w))
        s += w
    return out


def _build(L_c):
    """Build + compile the per-core Bass program for compacted key length L_c."""
    nc = bacc.Bacc("TRN2", debug=False, num_devices=NCORES)
    mts = _chunks(L_c, 128)
    MT = len(mts)
    EXP = mybir.ActivationFunctionType.Exp

    xb_d = nc.declare_dram_parameter("xb", [D, L], BF16, isOutput=False)
    xk_d = nc.declare_dram_parameter("xk", [D, L_c], BF16, isOutput=False)
    vo_d = nc.declare_dram_parameter("vones", [L_c, HPC, 1], F32, isOutput=False)
    wq_d = nc.declare_dram_parameter("wq", [D, DH], BF16, isOutput=False)
    wk_d = nc.declare_dram_parameter("wk", [D, DH], BF16, isOutput=False)
    wv_d = nc.declare_dram_parameter("wv", [D, DH], BF16, isOutput=False)
    wo_d = nc.declare_dram_parameter("wo", [DH, D], BF16, isOutput=False)
    bias_d = nc.declare_dram_parameter("bias", [128, 6], F32, isOutput=False)
    out_d = nc.declare_dram_parameter("out", [D, L], BF16, isOutput=True)

    from contextlib import ExitStack
    with tile.TileContext(nc) as tc, ExitStack() as ctx:
        pers = ctx.enter_context(tc.tile_pool(name="pers", bufs=1))

        def ptile(shape, dtype, name):
            return pers.tile(shape, dtype, tag=name, name=name)

        # persistent SBUF tensors
        xk_t = [ptile([128, L_c], BF16, f"xk{i}") for i in range(KT)]
        xb_t = [ptile([128, L], BF16, f"xb{i}") for i in range(KT)]
        wq_t = [ptile([128, DH], BF16, f"wq{i}") for i in range(KT)]
        wk_t = [ptile([128, DH], BF16, f"wk{i}") for i in range(KT)]
        wv_t = [ptile([128, DH], BF16, f"wv{i}") for i in range(KT)]
        wo_t = [ptile([128, D], BF16, f"wo{i}") for i in range(2)]
        bias_all = ptile([128, 6], F32, "bias_all")
        bq_t = [bias_all[:, 3 * i + 0:3 * i + 1] for i in range(2)]
        bk_t = [bias_all[:, 3 * i + 1:3 * i + 2] for i in range(2)]
        bv_t = [bias_all[:, 3 * i + 2:3 * i + 3] for i in range(2)]
        vo_all = ptile([128, MT, HPC, 1], F32, "vo_all")
        vo_t = [vo_all[0:mw, mt] for mt, (ms, mw) in enumerate(mts)]
        q_t = [ptile([128, L], BF16, f"q{i}") for i in range(2)]
        k_t = [ptile([128, L_c], BF16, f"k{i}") for i in range(2)]
        z_t = [ptile([128, L], BF16, f"z{i}") for i in range(2)]
        va_t = [ptile([mw, HPC, 65], BF16, f"va{mt}") for mt, (ms, mw) in enumerate(mts)]

        # input DMAs, critical-path order: the K projection chain consumes
        # (xk_i, wk_i) pairs in i order, so interleave those; then wq and the
        # first 512-column slice of xb (all the first Q chain needs), then the
        # rest of xb, then the late-use tensors (wv, vones, wo).
        for i in range(KT):
            nc.sync.dma_start(xk_t[i][:], xk_d[i * 128:(i + 1) * 128, :])
            nc.sync.dma_start(wk_t[i][:], wk_d[i * 128:(i + 1) * 128, :])
        nc.sync.dma_start(bias_all[:], bias_d[:])
        for i in range(KT):
            nc.sync.dma_start(wq_t[i][:], wq_d[i * 128:(i + 1) * 128, :])
            nc.sync.dma_start(xb_t[i][:, 0:512], xb_d[i * 128:(i + 1) * 128, 0:512])
        for js in (512, 1024, 1536):
            for i in range(KT):
                nc.sync.dma_start(
                    xb_t[i][:, js:js + 512], xb_d[i * 128:(i + 1) * 128, js:js + 512]
                )
        for i in range(KT):
            nc.sync.dma_start(wv_t[i][:], wv_d[i * 128:(i + 1) * 128, :])
        nc.sync.dma_start(
            vo_all[0:128, 0:MT - 1],
            vo_d[0:128 * (MT - 1)].rearrange("(t p) c u -> p t c u", p=128),
        )
        lms, lmw = mts[-1]
        nc.sync.dma_start(vo_all[0:lmw, MT - 1], vo_d[lms:lms + lmw])
        for i in range(2):
            nc.sync.dma_start(wo_t[i][:], wo_d[i * 128:(i + 1) * 128, :])

        with (
            tc.tile_pool(name="psA", bufs=2, space="PSUM") as pa,
            tc.tile_pool(name="psY", bufs=2, space="PSUM") as pb,
            tc.tile_pool(name="psO", bufs=2, space="PSUM") as pox,
            tc.tile_pool(name="pexp", bufs=2 * MT + 4) as pp,
            tc.tile_pool(name="osb", bufs=3) as po,
            tc.tile_pool(name="small", bufs=3) as psm,
        ):
            qblocks = _chunks(L, 1024)

            def k_chain(kt, ns, nw):
                kp = pox.tile([128, nw], F32, tag="po", name=f"kp{kt}_{ns}")
                for kk in range(KT):
                    nc.tensor.matmul(
                        kp[:],
                        wk_t[kk][:, kt * 128:(kt + 1) * 128],
                        xk_t[kk][:, ns:ns + nw],
                        start=(kk == 0), stop=(kk == KT - 1),
                    )
                nc.vector.tensor_scalar_add(k_t[kt][:, ns:ns + nw], kp[:], bk_t[kt][:])

            def q_chain(qs, kt, js, jw):
                qp = pox.tile([128, jw], F32, tag="po", name=f"qp{kt}_{qs + js}")
                for kk in range(KT):
                    nc.tensor.matmul(
                        qp[:],
                        wq_t[kk][:, kt * 128:(kt + 1) * 128],
                        xb_t[kk][:, qs + js:qs + js + jw],
                        start=(kk == 0), stop=(kk == KT - 1),
                    )
                nc.vector.tensor_scalar_add(q_t[kt][:, qs + js:qs + js + jw], qp[:], bq_t[kt][:])

            def v_chain(mt):
                ms, mw = mts[mt]
                vp = pox.tile([mw, DH], F32, tag="po", name=f"vp{mt}")
                for kk in range(KT):
                    nc.tensor.matmul(
                        vp[:],
                        xk_t[kk][:, ms:ms + mw],
                        wv_t[kk][:],
                        start=(kk == 0), stop=(kk == KT - 1),
                    )
                for h in range(HPC):
                    nc.vector.tensor_copy(va_t[mt][:, h, 0:64], vp[:, h * 64:(h + 1) * 64])
                nc.vector.tensor_copy(va_t[mt][:, :, 64:65], vo_t[mt][:])

            def o_chunk(qs, m8, js, jw):
                op = pox.tile([128, jw], F32, tag="po", name=f"o{qs}_{m8}_{js}")
                for kt in range(2):
                    nc.tensor.matmul(
                        op[:],
                        wo_t[kt][:, m8 * 128:(m8 + 1) * 128],
                        z_t[kt][:, qs + js:qs + js + jw],
                        start=(kt == 0), stop=(kt == 1),
                    )
                ob = po.tile([128, jw], BF16, tag="ob", name=f"ob{qs}_{m8}_{js}")
                nc.vector.tensor_copy(ob[:], op[:])
                nc.sync.dma_start(out_d[m8 * 128:(m8 + 1) * 128, qs + js:qs + js + jw], ob[:])

            # minimal prologue: just enough K/Q for head 0's first scores
            kchunks = _chunks(L_c, 512)
            k_chain(0, *kchunks[0])
            q_chain(0, 0, 0, 512)
            q_chain(0, 0, 512, 512)

            # ---- software-pipelined attention, head-PAIR phases ----
            # The two heads of a ptile row-pack the PE array: their K=64 score
            # matmuls are emitted strictly alternating (rows 0-63 vs 64-127)
            # so the array streams both concurrently (~2x). The previous
            # pair's att@v chains + projections/O-chunks fill remaining PE
            # slots while ACT streams the exps.
            def y_head(h, qs, qw, p_tiles, yq):
                # enqueue one head's y chains (js-interleaved, 2 PSUM banks)
                yps = {}

                def y_mt(mt):
                    if mt == 0:
                        for js, jw in _chunks(qw, 512):
                            yps[js] = pb.tile([65, jw], F32, tag="y", name=f"y{qs}_{h}_{js}")
                    for js, jw in _chunks(qw, 512):
                        nc.tensor.matmul(
                            yps[js],
                            va_t[mt][:, h, :],
                            p_tiles[mt][:, js:js + jw],
                            start=(mt == 0), stop=(mt == MT - 1),
                        )

                def finish():
                    pt, off = h // 2, (h % 2) * 64
                    for js, jw in _chunks(qw, 512):
                        yp = yps[js]
                        rt = psm.tile([1, jw], F32, tag="rrow", name=f"rt{qs}_{h}_{js}")
                        nc.vector.tensor_copy(rt[:], yp[64:65, :])
                        rc = psm.tile([1, jw], F32, tag="recip", name=f"rc{qs}_{h}_{js}")
                        nc.vector.reciprocal_approx_fast(rc[:], rt[:])
                        rb = psm.tile([64, jw], F32, tag="rb", name=f"rb{qs}_{h}_{js}")
                        nc.gpsimd.partition_broadcast(rb[:], rc[:])
                        zsl = z_t[pt][off:off + 64, qs + js:qs + js + jw]
                        nc.vector.tensor_mul(zsl, yp[0:64, :], rb[:])
                        nc.vector.tensor_scalar_add(zsl, zsl, bv_t[pt][off:off + 64, :])

                for mt in range(MT):
                    yq.append(lambda mt=mt: y_mt(mt))
                yq.append(finish)

            fillers = []   # (cost, emit) pairs
            fi = 0

            def pop_fillers(budget):
                nonlocal fi
                while budget > 0 and fi < len(fillers):
                    cost, emit = fillers[fi]
                    emit()
                    fi += 1
                    budget -= cost
                return budget

            for ns, nw in kchunks[1:]:
                fillers.append((8, lambda ns=ns, nw=nw: k_chain(0, ns, nw)))
            for ns, nw in kchunks:
                fillers.append((8, lambda ns=ns, nw=nw: k_chain(1, ns, nw)))
            for js, jw in _chunks(1024, 512):
                fillers.append((8, lambda js=js, jw=jw: q_chain(0, 1, js, jw)))
            for kt in range(2):
                for js, jw in _chunks(1024, 512):
                    fillers.append((8, lambda kt=kt, js=js, jw=jw: q_chain(1024, kt, js, jw)))

            qblocks = _chunks(L, 1024)
            yq = []       # pending y work units of the previous pair

            for qi, (qs, qw) in enumerate(qblocks):
                for hp in range(2):
                    hA, hB = 2 * hp, 2 * hp + 1
                    off_pairs = ((0, hA), (64, hB))
                    first_phase = (qi, hp) == (0, 0)
                    if (qi, hp) == (1, 1):
                        # z of block 0 completed during the previous phase:
                        # its O-projection chunks become filler work now
                        pqs, pqw = qblocks[0]
                        for m8 in range(8):
                            for js, jw in _chunks(pqw, 512):
                                fillers.append((2, lambda pqs=pqs, m8=m8, js=js, jw=jw: o_chunk(pqs, m8, js, jw)))
                    pA, pB = [], []
                    for mt, (ms, mw) in enumerate(mts):
                        sps = [
                            pa.tile([mw, qw], F32, tag="wide", name=f"s{qs}_{h}_{mt}")
                            for off, h in off_pairs
                        ]
                        for js, jw in _chunks(qw, 512):
                            for (off, h), sp in zip(off_pairs, sps):
                                nc.tensor.matmul(
                                    sp[:, js:js + jw],
                                    k_t[hp][off:off + 64, ms:ms + mw],
                                    q_t[hp][off:off + 64, qs + js:qs + js + jw],
                                    start=True, stop=True,
                                )
                        for (off, h), sp in zip(off_pairs, sps):
                            px = pp.tile([mw, qw], BF16, tag="p", name=f"p{qs}_{h}_{mt}")
                            nc.scalar.activation(px[:], sp[:], EXP)
                            (pA if h == hA else pB).append(px)
                        # drain previous pair's y work + filler budget
                        for _ in range(3):
                            if yq:
                                yq.pop(0)()
                        if first_phase:
                            pop_fillers(8)
                            v_chain(mt)
                        else:
                            pop_fillers(4)
                    while yq:
                        yq.pop(0)()
                    yq = []
                    y_head(hA, qs, qw, pA, yq)
                    y_head(hB, qs, qw, pB, yq)

            # drain the final pair's y work, remaining fillers, final O block
            while yq:
                yq.pop(0)()
            while fi < len(fillers):
                pop_fillers(1000)
            qs, qw = qblocks[-1]
            for m8 in range(8):
                for js, jw in _chunks(qw, 512):
                    o_chunk(qs, m8, js, jw)

    nc.compile()
    return nc


_NC_CACHE = {}


def _get_nc(L_c):
    if L_c not in _NC_CACHE:
        _NC_CACHE[L_c] = _build(L_c)
    return _NC_CACHE[L_c]


def _install_ntff_hook():
    """Synthesize antenv.axon_hooks (missing in this image) so trace=True works."""
    import types

    if "antenv.axon_hooks" in sys.modules:
        return
    try:
        if "/root/.axon_site" not in sys.path:
            sys.path.insert(0, "/root/.axon_site")
        from trn_agent_boot.trn_boot import _ntff_profile_via_ctypes

        hook = _ntff_profile_via_ctypes("/opt/axon/libaxon_pjrt.so")
        mod = types.ModuleType("antenv.axon_hooks")
        mod.get_axon_ntff_profile_hook = lambda: hook
        import antenv  # noqa: F401

        sys.modules["antenv.axon_hooks"] = mod
    except Exception:
        pass


def kernel(query, att_mask, Wq, bq, Wk, bk, Wv, bv, Wo, bo):
    global LAST_EXEC_NS, LAST_RESULTS
    query = np.asarray(query, dtype=np.float32)
    mask = np.asarray(att_mask).astype(bool).reshape(B, L)
    Wq, bq = np.asarray(Wq, np.float32), np.asarray(bq, np.float32)
    Wk, bk = np.asarray(Wk, np.float32), np.asarray(bk, np.float32)
    Wv, bv = np.asarray(Wv, np.float32), np.asarray(bv, np.float32)
    Wo, bo = np.asarray(Wo, np.float32), np.asarray(bo, np.float32)

    valid = [np.nonzero(mask[b])[0] for b in range(B)]
    L_c = max(len(v) for v in valid)
    out = np.empty((B, D, L), np.float32)
    if L_c == 0:
        out[:] = bo[None, :, None]
        return out

    scale = np.float32(1.0 / np.sqrt(DK))
    # per-batch compacted keys + validity column
    xk_b, vones_b, xb_b = [], [], []
    for b in range(B):
        idx = valid[b]
        xk = np.zeros((D, L_c), np.float32)
        xk[:, :len(idx)] = query[b][:, idx]
        xk_b.append(xk.astype(NPBF16))
        vo = np.zeros((L_c, HPC, 1), np.float32)
        vo[:len(idx)] = 1.0
        vones_b.append(vo)
        xb_b.append(query[b].astype(NPBF16))

    in_maps = []
    for c in range(NCORES):
        b, g = divmod(c, NCORES // B)
        sl = slice(g * DH, (g + 1) * DH)
        in_maps.append({
            "xb": xb_b[b],
            "xk": xk_b[b],
            "vones": vones_b[b],
            "wq": np.ascontiguousarray((Wq[sl, :] * scale).T).astype(NPBF16),
            "wk": np.ascontiguousarray(Wk[sl, :].T).astype(NPBF16),
            "wv": np.ascontiguousarray(Wv[sl, :].T).astype(NPBF16),
            "wo": np.ascontiguousarray(Wo[:, sl].T).astype(NPBF16),
            "bias": np.stack(
                [(bq[sl] * scale), bk[sl], bv[sl]], axis=-1
            ).reshape(2, 128, 3).transpose(1, 0, 2).reshape(128, 6).astype(np.float32),
        })

    nc = _get_nc(L_c)
    if TRACE:
        _install_ntff_hook()
    res = run_bass_kernel_spmd(nc, in_maps, core_ids=list(range(NCORES)), trace=TRACE)
    LAST_EXEC_NS = res.exec_time_ns
    LAST_RESULTS = res

    parts = [res.results[c]["out"] for c in range(NCORES)]
    for b in range(B):
        if len(valid[b]) == 0:
            out[b] = bo[:, None]
        else:
            acc = parts[4 * b].astype(np.float32)
            for g in range(1, 4):
                acc = acc + parts[4 * b + g]
            out[b] = acc + bo[:, None]
    return out
